# revision 3
# baseline (speedup 1.0000x reference)
"""Trainium2 Bass kernel for nn_BidirectionalNeuralSymbolic (vq_codebook).

Model (per batch row x of dim 1024):
  encoded = relu(x @ W1 + b1) @ W2 + b2                      # [E=128]
  sims    = cos(encoded, protos[k]) for k in 4096            # [K]
  concept_probs = softmax(10 * sims)                         # output 1 [B,K]
  best = argmax(concept_probs); bp = protos[best]            # gather
  ai = [encoded, bp]                                         # [256]
  abstraction = relu(ai @ W3 + b3) @ W4 + b4                 # output 2 [B,E]
  hierarchy   = sigmoid(ai @ Wh.T + bh)                      # output 3 [B,L]

Sharding: pure data-parallel over batch across 8 NeuronCores; all params
replicated. Each core handles 4096 rows as 32 tiles of 128.

Per-core dataflow (per 128-row tile, fp32 throughout):
  - PE-transpose x tile (8x 128x128) -> feature-major x^T
  - H1 (feature-major) via 32 accumulating matmuls; ACT fused bias+ReLU
  - encoded^T via 4 matmuls; ACT fused bias
  - row norms: ACT Square(+accum via ones-matmul) -> rsqrt via
    exp(-0.5*ln(s)) + one Newton step (single ACT table set: ln/exp)
  - sims = encoded^T.T @ pro_nT (protos pre-normalized in prologue);
    enc-norm * 10 folded into the per-partition ACT scale of the exp
  - softmax: ACT Exp writes exp-values + accum_out row-sums; DVE
    reciprocal + tensor_scalar multiply
  - argmax: DVE max8 + max_index (first-index ties, like jnp.argmax);
    indirect-DMA gather of protos rows
  - abstraction / hierarchy: small matmuls; sigmoid = 1/(1+exp(-z))
    so the whole kernel uses ONE activation table set.
"""

import numpy as np
from contextlib import ExitStack

import concourse.bass as bass
import concourse.mybir as mybir
import concourse.tile as tile
from concourse import bacc
from concourse.bass_utils import run_bass_kernel_spmd
from concourse.masks import make_identity

F32 = mybir.dt.float32
U32 = mybir.dt.uint32
AF = mybir.ActivationFunctionType
OP = mybir.AluOpType

B, D_IN, K, E, L, H = 32768, 1024, 4096, 128, 5, 512
N_CORES = 8
BC = B // N_CORES          # 4096 rows per core
P = 128
NT = BC // P               # 32 tiles per core
KC = D_IN // P             # 8 contraction chunks for layer 1
HC = H // P                # 4 hidden chunks
SC = 8                     # sims chunks of 512
SW = K // SC               # 512

_cached = None
last_results = None


def _build():
    nc = bacc.Bacc("TRN2", target_bir_lowering=False, debug=False)

    x_d = nc.dram_tensor("inputs", [BC, D_IN], F32, kind="ExternalInput")
    w1_d = nc.dram_tensor("W1", [D_IN, H], F32, kind="ExternalInput")
    b1_d = nc.dram_tensor("b1", [H], F32, kind="ExternalInput")
    w2_d = nc.dram_tensor("W2", [H, E], F32, kind="ExternalInput")
    b2_d = nc.dram_tensor("b2", [E], F32, kind="ExternalInput")
    pr_d = nc.dram_tensor("protos", [K, E], F32, kind="ExternalInput")
    w3_d = nc.dram_tensor("W3", [2 * E, E], F32, kind="ExternalInput")
    b3_d = nc.dram_tensor("b3", [E], F32, kind="ExternalInput")
    w4_d = nc.dram_tensor("W4", [E, E], F32, kind="ExternalInput")
    b4_d = nc.dram_tensor("b4", [E], F32, kind="ExternalInput")
    wh_d = nc.dram_tensor("Wh", [L, 2 * E], F32, kind="ExternalInput")
    bh_d = nc.dram_tensor("bh", [L], F32, kind="ExternalInput")

    probs_d = nc.dram_tensor("probs", [BC, K], F32, kind="ExternalOutput")
    abst_d = nc.dram_tensor("abst", [BC, E], F32, kind="ExternalOutput")
    hier_d = nc.dram_tensor("hier", [BC, L], F32, kind="ExternalOutput")

    with tile.TileContext(nc) as tc, ExitStack() as ctx:
        const = ctx.enter_context(tc.tile_pool(name="const", bufs=1))
        work = ctx.enter_context(tc.tile_pool(name="work", bufs=2))
        xpool = ctx.enter_context(tc.tile_pool(name="xpool", bufs=3))
        bigp = ctx.enter_context(tc.tile_pool(name="bigp", bufs=2))
        ps_xt = ctx.enter_context(tc.tile_pool(name="ps_xt", bufs=1, space="PSUM"))
        ps_sm = ctx.enter_context(tc.tile_pool(name="ps_sm", bufs=3, space="PSUM"))
        ps_si = ctx.enter_context(tc.tile_pool(name="ps_si", bufs=3, space="PSUM"))

        # ---------------- prologue: constants ----------------
        ident = const.tile([P, P], F32)
        make_identity(nc, ident[:])
        ones = const.tile([P, 1], F32)
        nc.vector.memset(ones[:], 1.0)

        w1sb = const.tile([P, KC * H], F32)       # block kc at [:, kc*512:...]
        for kc in range(KC):
            nc.sync.dma_start(
                w1sb[:, kc * H:(kc + 1) * H], w1_d[kc * P:(kc + 1) * P, :]
            )
        w2sb = const.tile([P, HC * E], F32)
        for hc in range(HC):
            nc.sync.dma_start(
                w2sb[:, hc * E:(hc + 1) * E], w2_d[hc * P:(hc + 1) * P, :]
            )
        w3sb = const.tile([P, 2 * E], F32)
        for c in range(2):
            nc.sync.dma_start(
                w3sb[:, c * E:(c + 1) * E], w3_d[c * P:(c + 1) * P, :]
            )
        w4sb = const.tile([P, E], F32)
        nc.sync.dma_start(w4sb[:], w4_d[:])
        whT = const.tile([P, 2 * L], F32)         # chunk c at [:, c*L:(c+1)*L]
        for c in range(2):
            nc.sync.dma_start(
                whT[:, c * L:(c + 1) * L],
                wh_d[:, c * P:(c + 1) * P].rearrange("l p -> p l"),
            )
        b1c = const.tile([P, HC], F32)
        nc.sync.dma_start(b1c[:], b1_d.ap().rearrange("(c p) -> p c", p=P))
        b2c = const.tile([P, 1], F32)
        nc.sync.dma_start(b2c[:], b2_d.ap().rearrange("(c p) -> p c", p=P))
        b3c = const.tile([P, 1], F32)
        nc.sync.dma_start(b3c[:], b3_d.ap().rearrange("(c p) -> p c", p=P))
        b4row = const.tile([1, E], F32)
        nc.sync.dma_start(b4row[:], b4_d.ap().rearrange("(o f) -> o f", o=1))
        b4b = const.tile([P, E], F32)
        nc.gpsimd.partition_broadcast(b4b[:], b4row[:])
        bhrow = const.tile([1, L], F32)
        nc.sync.dma_start(bhrow[:], bh_d.ap().rearrange("(o f) -> o f", o=1))
        bhb = const.tile([P, L], F32)
        nc.gpsimd.partition_broadcast(bhb[:], bhrow[:])

        # normalized-protos^T  [E, K]
        pro_nT = const.tile([P, K], F32)
        for c in range(K // P):
            pb = work.tile([P, P], F32, tag="pb")
            nc.sync.dma_start(pb[:], pr_d[c * P:(c + 1) * P, :])
            psq = work.tile([P, P], F32, tag="psq")
            ssq = work.tile([P, 1], F32, tag="ssq")
            nc.scalar.activation(psq[:], pb[:], AF.Square, accum_out=ssq[:])
            lnt = work.tile([P, 1], F32, tag="lnt")
            nc.scalar.activation(lnt[:], ssq[:], AF.Ln)
            y0 = work.tile([P, 1], F32, tag="y0")
            nc.scalar.activation(y0[:], lnt[:], AF.Exp, scale=-0.5)
            # Newton polish: y1 = y0*(1.5 - 0.5*s*y0^2), clamped at 1e8
            y0sq = work.tile([P, 1], F32, tag="y0sq")
            nc.vector.tensor_tensor(out=y0sq[:], in0=y0[:], in1=y0[:], op=OP.mult)
            tt = work.tile([P, 1], F32, tag="tt")
            nc.vector.tensor_tensor(out=tt[:], in0=y0sq[:], in1=ssq[:], op=OP.mult)
            tt2 = work.tile([P, 1], F32, tag="tt2")
            nc.vector.tensor_scalar(
                out=tt2[:], in0=tt[:], scalar1=-0.5, scalar2=1.5,
                op0=OP.mult, op1=OP.add,
            )
            ipn = work.tile([P, 1], F32, tag="ipn")
            nc.vector.tensor_tensor(out=ipn[:], in0=y0[:], in1=tt2[:], op=OP.mult)
            ipc = work.tile([P, 1], F32, tag="ipc")
            nc.vector.tensor_scalar(
                out=ipc[:], in0=ipn[:], scalar1=1e8, scalar2=None, op0=OP.min,
            )
            pn = work.tile([P, P], F32, tag="pn")
            nc.vector.tensor_scalar(
                out=pn[:], in0=pb[:], scalar1=ipc[:, :1], scalar2=None, op0=OP.mult,
            )
            pt_ps = ps_sm.tile([P, P], F32, tag="sm")
            nc.tensor.transpose(pt_ps[:], pn[:], ident[:])
            nc.scalar.copy(pro_nT[:, c * P:(c + 1) * P], pt_ps[:])

        # ---------------- main loop over 32 batch tiles ----------------
        for t in range(NT):
            row0 = t * P
            xf = xpool.tile([P, D_IN], F32, tag="xf")
            nc.sync.dma_start(xf[:], x_d[row0:row0 + P, :])

            # x^T (feature-major) via 8 PE transposes
            xt_ps = ps_xt.tile([P, D_IN], F32, tag="xt")
            for kc in range(KC):
                nc.tensor.transpose(
                    xt_ps[:, kc * P:(kc + 1) * P], xf[:, kc * P:(kc + 1) * P],
                    ident[:],
                )
            xt = bigp.tile([P, D_IN], F32, tag="xt_sb")
            nc.scalar.copy(xt[:], xt_ps[:])

            # H1^T = relu(W1^T x^T + b1): 4 chunks of [128h, 128b]
            h1_ps = ps_sm.tile([P, H], F32, tag="sm")
            for hc in range(HC):
                for kc in range(KC):
                    nc.tensor.matmul(
                        h1_ps[:, hc * P:(hc + 1) * P],
                        w1sb[:, kc * H + hc * P: kc * H + (hc + 1) * P],
                        xt[:, kc * P:(kc + 1) * P],
                        start=(kc == 0), stop=(kc == KC - 1),
                    )
            h1 = work.tile([P, H], F32, tag="h1")
            for hc in range(HC):
                nc.scalar.activation(
                    h1[:, hc * P:(hc + 1) * P], h1_ps[:, hc * P:(hc + 1) * P],
                    AF.Relu, bias=b1c[:, hc:hc + 1],
                )

            # encoded^T [E, b]
            e0_ps = ps_sm.tile([P, E], F32, tag="sm")
            for hc in range(HC):
                nc.tensor.matmul(
                    e0_ps[:], w2sb[:, hc * E:(hc + 1) * E],
                    h1[:, hc * P:(hc + 1) * P],
                    start=(hc == 0), stop=(hc == HC - 1),
                )
            e0 = work.tile([P, E], F32, tag="e0")
            nc.scalar.activation(e0[:], e0_ps[:], AF.Identity, bias=b2c[:, :1])

            # per-row ||encoded||^2 -> [b, 1] via ones-matmul
            e0sq = work.tile([P, E], F32, tag="e0sq")
            nc.scalar.activation(e0sq[:], e0[:], AF.Square)
            ns_ps = ps_sm.tile([P, 1], F32, tag="sm")
            nc.tensor.matmul(ns_ps[:], e0sq[:], ones[:], start=True, stop=True)
            # 10 / max(||e||, eps)  via exp(-0.5 ln s) + Newton
            lnt = work.tile([P, 1], F32, tag="m_ln")
            nc.scalar.activation(lnt[:], ns_ps[:], AF.Ln)
            y0 = work.tile([P, 1], F32, tag="m_y0")
            nc.scalar.activation(y0[:], lnt[:], AF.Exp, scale=-0.5)
            y0sq = work.tile([P, 1], F32, tag="m_y0sq")
            nc.vector.tensor_tensor(out=y0sq[:], in0=y0[:], in1=y0[:], op=OP.mult)
            tt = work.tile([P, 1], F32, tag="m_tt")
            nc.vector.tensor_tensor(out=tt[:], in0=y0sq[:], in1=ns_ps[:], op=OP.mult)
            tt2 = work.tile([P, 1], F32, tag="m_tt2")
            nc.vector.tensor_scalar(
                out=tt2[:], in0=tt[:], scalar1=-0.5, scalar2=1.5,
                op0=OP.mult, op1=OP.add,
            )
            y1 = work.tile([P, 1], F32, tag="m_y1")
            nc.vector.tensor_tensor(out=y1[:], in0=y0[:], in1=tt2[:], op=OP.mult)
            scl = work.tile([P, 1], F32, tag="m_scl")
            nc.vector.tensor_scalar(
                out=scl[:], in0=y1[:], scalar1=1e8, scalar2=10.0,
                op0=OP.min, op1=OP.mult,
            )

            # sims -> exp(10*invn*sims), chunk by chunk; accum row-sums
            ex = bigp.tile([P, K], F32, tag="ex")
            s8 = work.tile([P, SC], F32, tag="s8")
            for c in range(SC):
                si_ps = ps_si.tile([P, SW], F32, tag="si")
                nc.tensor.matmul(
                    si_ps[:], e0[:], pro_nT[:, c * SW:(c + 1) * SW],
                    start=True, stop=True,
                )
                nc.scalar.activation(
                    ex[:, c * SW:(c + 1) * SW], si_ps[:], AF.Exp,
                    scale=scl[:, :1], accum_out=s8[:, c:c + 1],
                )
            ssum = work.tile([P, 1], F32, tag="ssum")
            nc.vector.reduce_sum(out=ssum[:], in_=s8[:], axis=mybir.AxisListType.X)
            rs = work.tile([P, 1], F32, tag="rs")
            nc.vector.reciprocal(rs[:], ssum[:])
            pr = bigp.tile([P, K], F32, tag="pr")
            nc.vector.tensor_scalar(
                out=pr[:], in0=ex[:], scalar1=rs[:, :1], scalar2=None, op0=OP.mult,
            )
            nc.sync.dma_start(probs_d[row0:row0 + P, :], pr[:])

            # argmax over exp values (same order as probs) + gather protos row
            mx8 = work.tile([P, 8], F32, tag="mx8")
            nc.vector.max(out=mx8[:], in_=ex[:])
            mi8 = work.tile([P, 8], U32, tag="mi8")
            nc.vector.max_index(out=mi8[:], in_max=mx8[:], in_values=ex[:])
            bp = work.tile([P, E], F32, tag="bp")
            nc.gpsimd.indirect_dma_start(
                out=bp[:], out_offset=None, in_=pr_d[:],
                in_offset=bass.IndirectOffsetOnAxis(ap=mi8[:, :1], axis=0),
            )
            bpt_ps = ps_sm.tile([P, E], F32, tag="sm")
            nc.tensor.transpose(bpt_ps[:], bp[:], ident[:])
            bpt = work.tile([P, E], F32, tag="bpt")
            nc.scalar.copy(bpt[:], bpt_ps[:])

            # abstraction = relu(ai@W3+b3)@W4 + b4   (ai = [encoded, bp])
            a1_ps = ps_sm.tile([P, E], F32, tag="sm")
            nc.tensor.matmul(a1_ps[:], w3sb[:, :E], e0[:], start=True, stop=False)
            nc.tensor.matmul(a1_ps[:], w3sb[:, E:], bpt[:], start=False, stop=True)
            a1 = work.tile([P, E], F32, tag="a1")
            nc.scalar.activation(a1[:], a1_ps[:], AF.Relu, bias=b3c[:, :1])
            ab_ps = ps_sm.tile([P, E], F32, tag="sm")
            nc.tensor.matmul(ab_ps[:], a1[:], w4sb[:], start=True, stop=True)
            ab = work.tile([P, E], F32, tag="ab")
            nc.vector.tensor_tensor(out=ab[:], in0=ab_ps[:], in1=b4b[:], op=OP.add)
            nc.sync.dma_start(abst_d[row0:row0 + P, :], ab[:])

            # hierarchy = sigmoid(ai @ Wh^T + bh) = 1/(1+exp(-z))
            hr_ps = ps_sm.tile([P, L], F32, tag="sm")
            nc.tensor.matmul(hr_ps[:], e0[:], whT[:, :L], start=True, stop=False)
            nc.tensor.matmul(hr_ps[:], bpt[:], whT[:, L:], start=False, stop=True)
            hz = work.tile([P, L], F32, tag="hz")
            nc.vector.tensor_tensor(out=hz[:], in0=hr_ps[:], in1=bhb[:], op=OP.add)
            he = work.tile([P, L], F32, tag="he")
            nc.scalar.activation(he[:], hz[:], AF.Exp, scale=-1.0)
            hd = work.tile([P, L], F32, tag="hd")
            nc.vector.tensor_scalar(
                out=hd[:], in0=he[:], scalar1=1.0, scalar2=None, op0=OP.add,
            )
            hs = work.tile([P, L], F32, tag="hs")
            nc.vector.reciprocal(hs[:], hd[:])
            nc.sync.dma_start(hier_d[row0:row0 + P, :], hs[:])

    nc.compile()
    return nc


def kernel(**inputs):
    global _cached
    if _cached is None:
        _cached = _build()
    nc = _cached

    full = {k: np.ascontiguousarray(np.asarray(v, dtype=np.float32))
            for k, v in inputs.items()}
    shared = {k: v for k, v in full.items() if k != "inputs"}
    in_maps = []
    for c in range(N_CORES):
        m = dict(shared)
        m["inputs"] = np.ascontiguousarray(full["inputs"][c * BC:(c + 1) * BC])
        in_maps.append(m)

    global last_results
    res = run_bass_kernel_spmd(nc, in_maps, list(range(N_CORES)))
    last_results = res
    probs = np.concatenate([r["probs"] for r in res.results], axis=0)
    abst = np.concatenate([r["abst"] for r in res.results], axis=0)
    hier = np.concatenate([r["hier"] for r in res.results], axis=0)
    return probs, abst, hier


def run_traced(inputs):
    """Profiled run (test-harness helper; requires the axon NTFF hook)."""
    global _cached
    if _cached is None:
        _cached = _build()
    full = {k: np.ascontiguousarray(np.asarray(v, dtype=np.float32))
            for k, v in inputs.items()}
    shared = {k: v for k, v in full.items() if k != "inputs"}
    in_maps = []
    for c in range(N_CORES):
        m = dict(shared)
        m["inputs"] = np.ascontiguousarray(full["inputs"][c * BC:(c + 1) * BC])
        in_maps.append(m)
    return run_bass_kernel_spmd(_cached, in_maps, list(range(N_CORES)), trace=True)


# revision 5
# speedup vs baseline: 2.0281x; 2.0281x over previous
"""Trainium2 Bass kernel for nn_BidirectionalNeuralSymbolic (vq_codebook).

Model (per batch row x of dim 1024):
  encoded = relu(x @ W1 + b1) @ W2 + b2                      # [E=128]
  sims    = cos(encoded, protos[k]) for k in 4096            # [K]
  concept_probs = softmax(10 * sims)                         # output 1 [B,K]
  best = argmax(concept_probs); bp = protos[best]            # gather
  ai = [encoded, bp]                                         # [256]
  abstraction = relu(ai @ W3 + b3) @ W4 + b4                 # output 2 [B,E]
  hierarchy   = sigmoid(ai @ Wh.T + bh)                      # output 3 [B,L]

Sharding: pure data-parallel over batch across 8 NeuronCores; all params
replicated. Each core handles 4096 rows as 8 blocks of 512 (4 sub-tiles
of 128 rows each) so the heavy matmuls run with a 512-wide moving
operand.

Engine plan per 512-row block (fp32 throughout):
  PE : 32 transposes of x -> x^T; H1 (32 mm N=512); encoded^T (4 mm
       N=512); per-row-norm ones-matmuls; sims (32 mm N=512, lhsT =
       encoded^T); best-proto transposes; abstraction/hierarchy mms.
  ACT: PSUM evacuations fused with bias/ReLU; exp of sims with the
       per-row 10/||enc|| in the per-partition `scale` operand and
       accum_out producing softmax denominators for free. Only EXP-set
       functions are used -> a single ACT table load for the kernel.
  DVE: softmax scale (1/sum), max8 + max_index argmax (first-index tie
       semantics identical to jnp.argmax), small fixups.
  GPS: rsqrt via pow(s, -0.5) (Q7 vpowf), indirect-DMA gather of the
       argmax protos rows, partition broadcasts.
PSUM banks: tp 1 + h1 2 + e0/ns 1 + sims 3 + late 1 = 8.
"""

import numpy as np
from contextlib import ExitStack

import concourse.bass as bass
import concourse.mybir as mybir
import concourse.tile as tile
from concourse import bacc
from concourse.bass_utils import run_bass_kernel_spmd
from concourse.masks import make_identity

F32 = mybir.dt.float32
U32 = mybir.dt.uint32
AF = mybir.ActivationFunctionType
OP = mybir.AluOpType

B, D_IN, K, E, L, H = 32768, 1024, 4096, 128, 5, 512
N_CORES = 8
BC = B // N_CORES          # 4096 rows per core
P = 128
SUB = 4                    # 128-row sub-tiles per block
BLK = SUB * P              # 512 rows per block
NB = BC // BLK             # 8 blocks per core
KC = D_IN // P             # 8 contraction chunks for layer 1
HC = H // P                # 4 hidden chunks
SC = 8                     # sims chunks of 512 per sub-tile
SW = K // SC               # 512

_cached = None
last_results = None


def _build():
    nc = bacc.Bacc("TRN2", target_bir_lowering=False, debug=False)

    x_d = nc.dram_tensor("inputs", [BC, D_IN], F32, kind="ExternalInput")
    w1_d = nc.dram_tensor("W1", [D_IN, H], F32, kind="ExternalInput")
    b1_d = nc.dram_tensor("b1", [H], F32, kind="ExternalInput")
    w2_d = nc.dram_tensor("W2", [H, E], F32, kind="ExternalInput")
    b2_d = nc.dram_tensor("b2", [E], F32, kind="ExternalInput")
    pr_d = nc.dram_tensor("protos", [K, E], F32, kind="ExternalInput")
    w3_d = nc.dram_tensor("W3", [2 * E, E], F32, kind="ExternalInput")
    b3_d = nc.dram_tensor("b3", [E], F32, kind="ExternalInput")
    w4_d = nc.dram_tensor("W4", [E, E], F32, kind="ExternalInput")
    b4_d = nc.dram_tensor("b4", [E], F32, kind="ExternalInput")
    wh_d = nc.dram_tensor("Wh", [L, 2 * E], F32, kind="ExternalInput")
    bh_d = nc.dram_tensor("bh", [L], F32, kind="ExternalInput")

    probs_d = nc.dram_tensor("probs", [BC, K], F32, kind="ExternalOutput")
    abst_d = nc.dram_tensor("abst", [BC, E], F32, kind="ExternalOutput")
    hier_d = nc.dram_tensor("hier", [BC, L], F32, kind="ExternalOutput")

    with tile.TileContext(nc) as tc, ExitStack() as ctx:
        const = ctx.enter_context(tc.tile_pool(name="const", bufs=1))
        work = ctx.enter_context(tc.tile_pool(name="work", bufs=2))
        xpool = ctx.enter_context(tc.tile_pool(name="xpool", bufs=6))
        bigp = ctx.enter_context(tc.tile_pool(name="bigp", bufs=2))
        ps_tp = ctx.enter_context(tc.tile_pool(name="ps_tp", bufs=1, space="PSUM"))
        ps_h1 = ctx.enter_context(tc.tile_pool(name="ps_h1", bufs=2, space="PSUM"))
        ps_en = ctx.enter_context(tc.tile_pool(name="ps_en", bufs=1, space="PSUM"))
        ps_si = ctx.enter_context(tc.tile_pool(name="ps_si", bufs=3, space="PSUM"))
        ps_lt = ctx.enter_context(tc.tile_pool(name="ps_lt", bufs=1, space="PSUM"))

        # ---------------- prologue: constants ----------------
        ident = const.tile([P, P], F32)
        make_identity(nc, ident[:])
        ones = const.tile([P, 1], F32)
        nc.vector.memset(ones[:], 1.0)
        nhalf = const.tile([P, SUB], F32)
        nc.vector.memset(nhalf[:], -0.5)
        nhalf1 = const.tile([P, 1], F32)
        nc.vector.memset(nhalf1[:], -0.5)

        w1sb = const.tile([P, KC * H], F32)       # block kc at [:, kc*512:...]
        for kc in range(KC):
            nc.sync.dma_start(
                w1sb[:, kc * H:(kc + 1) * H], w1_d[kc * P:(kc + 1) * P, :]
            )
        w2sb = const.tile([P, HC * E], F32)
        for hc in range(HC):
            nc.sync.dma_start(
                w2sb[:, hc * E:(hc + 1) * E], w2_d[hc * P:(hc + 1) * P, :]
            )
        w3sb = const.tile([P, 2 * E], F32)
        for c in range(2):
            nc.sync.dma_start(
                w3sb[:, c * E:(c + 1) * E], w3_d[c * P:(c + 1) * P, :]
            )
        w4sb = const.tile([P, E], F32)
        nc.sync.dma_start(w4sb[:], w4_d[:])
        whT = const.tile([P, 2 * L], F32)         # chunk c at [:, c*L:(c+1)*L]
        for c in range(2):
            nc.sync.dma_start(
                whT[:, c * L:(c + 1) * L],
                wh_d[:, c * P:(c + 1) * P].rearrange("l p -> p l"),
            )
        b1c = const.tile([P, HC], F32)
        nc.sync.dma_start(b1c[:], b1_d.ap().rearrange("(c p) -> p c", p=P))
        b2c = const.tile([P, 1], F32)
        nc.sync.dma_start(b2c[:], b2_d.ap().rearrange("(c p) -> p c", p=P))
        b3c = const.tile([P, 1], F32)
        nc.sync.dma_start(b3c[:], b3_d.ap().rearrange("(c p) -> p c", p=P))
        b4row = const.tile([1, E], F32)
        nc.sync.dma_start(b4row[:], b4_d.ap().rearrange("(o f) -> o f", o=1))
        b4b = const.tile([P, E], F32)
        nc.gpsimd.partition_broadcast(b4b[:], b4row[:])
        bh4row = const.tile([1, SUB * L], F32)
        for s in range(SUB):
            nc.sync.dma_start(
                bh4row[:, s * L:(s + 1) * L],
                bh_d.ap().rearrange("(o f) -> o f", o=1),
            )
        bhb4 = const.tile([P, SUB * L], F32)
        nc.gpsimd.partition_broadcast(bhb4[:], bh4row[:])

        # normalized-protos^T  [E, K]
        pro_nT = const.tile([P, K], F32)
        for c in range(K // P):
            pb = work.tile([P, P], F32, tag="pb")
            nc.sync.dma_start(pb[:], pr_d[c * P:(c + 1) * P, :])
            psq = work.tile([P, P], F32, tag="psq")
            ssq = work.tile([P, 1], F32, tag="ssq")
            nc.scalar.activation(psq[:], pb[:], AF.Square, accum_out=ssq[:])
            ipn = work.tile([P, 1], F32, tag="ipn")
            nc.gpsimd.tensor_tensor(
                out=ipn[:], in0=ssq[:], in1=nhalf1[:], op=OP.pow
            )
            ipc = work.tile([P, 1], F32, tag="ipc")
            nc.vector.tensor_scalar(
                out=ipc[:], in0=ipn[:], scalar1=1e8, scalar2=None, op0=OP.min,
            )
            pn = work.tile([P, P], F32, tag="pn")
            nc.vector.tensor_scalar(
                out=pn[:], in0=pb[:], scalar1=ipc[:, :1], scalar2=None, op0=OP.mult,
            )
            pt_ps = ps_lt.tile([P, P], F32, tag="lt")
            nc.tensor.transpose(pt_ps[:], pn[:], ident[:])
            nc.scalar.copy(pro_nT[:, c * P:(c + 1) * P], pt_ps[:])

        # ---------------- main loop over 8 blocks of 512 rows ----------------
        for b in range(NB):
            r0 = b * BLK
            xs = []
            for s in range(SUB):
                xf = xpool.tile([P, D_IN], F32, tag="xf")
                nc.sync.dma_start(
                    xf[:], x_d[r0 + s * P:r0 + (s + 1) * P, :]
                )
                xs.append(xf)

            # x^T feature-major: xt[:, kc*512 + s*128 + j] = x[s*128+i, kc*128+j]
            xt = bigp.tile([P, KC * BLK], F32, tag="xt")
            for kc in range(KC):
                tp_ps = ps_tp.tile([P, BLK], F32, tag="tp")
                for s in range(SUB):
                    nc.tensor.transpose(
                        tp_ps[:, s * P:(s + 1) * P],
                        xs[s][:, kc * P:(kc + 1) * P],
                        ident[:],
                    )
                nc.scalar.copy(xt[:, kc * BLK:(kc + 1) * BLK], tp_ps[:])

            # H1^T = relu(W1^T x^T + b1): 4 chunks of [128h, 512b]
            h1 = bigp.tile([P, HC * BLK], F32, tag="h1")
            for hc in range(HC):
                h1_ps = ps_h1.tile([P, BLK], F32, tag="h1p")
                for kc in range(KC):
                    nc.tensor.matmul(
                        h1_ps[:],
                        w1sb[:, kc * H + hc * P: kc * H + (hc + 1) * P],
                        xt[:, kc * BLK:(kc + 1) * BLK],
                        start=(kc == 0), stop=(kc == KC - 1),
                    )
                nc.scalar.activation(
                    h1[:, hc * BLK:(hc + 1) * BLK], h1_ps[:],
                    AF.Relu, bias=b1c[:, hc:hc + 1],
                )

            # encoded^T [E, 512b]
            e0_ps = ps_en.tile([P, BLK], F32, tag="en")
            for hc in range(HC):
                nc.tensor.matmul(
                    e0_ps[:], w2sb[:, hc * E:(hc + 1) * E],
                    h1[:, hc * BLK:(hc + 1) * BLK],
                    start=(hc == 0), stop=(hc == HC - 1),
                )
            e0 = work.tile([P, BLK], F32, tag="e0")
            nc.scalar.activation(e0[:], e0_ps[:], AF.Identity, bias=b2c[:, :1])

            # per-row norms: [b,1] per sub via ones-matmul, packed [128, SUB]
            e0sq = work.tile([P, BLK], F32, tag="e0sq")
            nc.scalar.activation(e0sq[:], e0[:], AF.Square)
            ns_ps = ps_en.tile([P, SUB], F32, tag="en")
            for s in range(SUB):
                nc.tensor.matmul(
                    ns_ps[:, s:s + 1], e0sq[:, s * P:(s + 1) * P], ones[:],
                    start=True, stop=True,
                )
            ns = work.tile([P, SUB], F32, tag="ns")
            nc.vector.tensor_copy(ns[:], ns_ps[:])
            # scale = 10 * min(pow(s, -0.5), 1e8)
            inv = work.tile([P, SUB], F32, tag="inv")
            nc.gpsimd.tensor_tensor(out=inv[:], in0=ns[:], in1=nhalf[:], op=OP.pow)
            scl = work.tile([P, SUB], F32, tag="scl")
            nc.vector.tensor_scalar(
                out=scl[:], in0=inv[:], scalar1=1e8, scalar2=10.0,
                op0=OP.min, op1=OP.mult,
            )

            for s in range(SUB):
                sr0 = r0 + s * P
                # sims -> exp(scale * sims); accum_out -> chunk sums
                ex = bigp.tile([P, K], F32, tag="ex")
                s8 = work.tile([P, SC], F32, tag="s8")
                for c in range(SC):
                    si_ps = ps_si.tile([P, SW], F32, tag="si")
                    nc.tensor.matmul(
                        si_ps[:], e0[:, s * P:(s + 1) * P],
                        pro_nT[:, c * SW:(c + 1) * SW],
                        start=True, stop=True,
                    )
                    nc.scalar.activation(
                        ex[:, c * SW:(c + 1) * SW], si_ps[:], AF.Exp,
                        scale=scl[:, s:s + 1], accum_out=s8[:, c:c + 1],
                    )
                ssum = work.tile([P, 1], F32, tag="ssum")
                nc.vector.reduce_sum(
                    out=ssum[:], in_=s8[:], axis=mybir.AxisListType.X
                )
                rs = work.tile([P, 1], F32, tag="rs")
                nc.vector.reciprocal(rs[:], ssum[:])

                # argmax over exp values (same ordering as probs)
                mx8 = work.tile([P, 8], F32, tag="mx8")
                nc.vector.max(out=mx8[:], in_=ex[:])
                mi8 = work.tile([P, 8], U32, tag="mi8")
                nc.vector.max_index(out=mi8[:], in_max=mx8[:], in_values=ex[:])
                bp = work.tile([P, E], F32, tag="bp")
                nc.gpsimd.indirect_dma_start(
                    out=bp[:], out_offset=None, in_=pr_d[:],
                    in_offset=bass.IndirectOffsetOnAxis(ap=mi8[:, :1], axis=0),
                )

                # probs = ex / sum  (in place), then DMA out
                nc.vector.tensor_scalar(
                    out=ex[:], in0=ex[:], scalar1=rs[:, :1], scalar2=None,
                    op0=OP.mult,
                )
                nc.sync.dma_start(probs_d[sr0:sr0 + P, :], ex[:])

                # best-protos^T for this sub-tile
                bpt_ps = ps_lt.tile([P, P], F32, tag="lt")
                nc.tensor.transpose(bpt_ps[:], bp[:], ident[:])
                bpts = work.tile([P, E], F32, tag="bpts")
                nc.scalar.copy(bpts[:], bpt_ps[:])

                # abstraction: A1^T = relu(W3^T ai^T + b3) -> [a1, 128b]
                a1_ps = ps_lt.tile([P, E], F32, tag="lt")
                nc.tensor.matmul(
                    a1_ps[:], w3sb[:, :E], e0[:, s * P:(s + 1) * P],
                    start=True, stop=False,
                )
                nc.tensor.matmul(
                    a1_ps[:], w3sb[:, E:], bpts[:], start=False, stop=True
                )
                a1 = work.tile([P, E], F32, tag="a1")
                nc.scalar.activation(a1[:], a1_ps[:], AF.Relu, bias=b3c[:, :1])
                ab_ps = ps_lt.tile([P, E], F32, tag="lt")
                nc.tensor.matmul(ab_ps[:], a1[:], w4sb[:], start=True, stop=True)
                ab = work.tile([P, E], F32, tag="ab")
                nc.vector.tensor_tensor(
                    out=ab[:], in0=ab_ps[:], in1=b4b[:], op=OP.add
                )
                nc.sync.dma_start(abst_d[sr0:sr0 + P, :], ab[:])

                # hierarchy = sigmoid(ai @ Wh^T + bh)
                hr_ps = ps_lt.tile([P, L], F32, tag="lt")
                nc.tensor.matmul(
                    hr_ps[:], e0[:, s * P:(s + 1) * P], whT[:, :L],
                    start=True, stop=False,
                )
                nc.tensor.matmul(
                    hr_ps[:], bpts[:], whT[:, L:], start=False, stop=True
                )
                hz = work.tile([P, L], F32, tag="hz")
                nc.vector.tensor_tensor(
                    out=hz[:], in0=hr_ps[:], in1=bhb4[:, :L], op=OP.add
                )
                he = work.tile([P, L], F32, tag="he")
                nc.scalar.activation(he[:], hz[:], AF.Exp, scale=-1.0)
                hd = work.tile([P, L], F32, tag="hd")
                nc.vector.tensor_scalar(
                    out=hd[:], in0=he[:], scalar1=1.0, scalar2=None, op0=OP.add,
                )
                hs = work.tile([P, L], F32, tag="hs")
                nc.vector.reciprocal(hs[:], hd[:])
                nc.sync.dma_start(hier_d[sr0:sr0 + P, :], hs[:])

    nc.compile()
    return nc


def kernel(**inputs):
    global _cached, last_results
    if _cached is None:
        _cached = _build()
    nc = _cached

    full = {k: np.ascontiguousarray(np.asarray(v, dtype=np.float32))
            for k, v in inputs.items()}
    shared = {k: v for k, v in full.items() if k != "inputs"}
    in_maps = []
    for c in range(N_CORES):
        m = dict(shared)
        m["inputs"] = np.ascontiguousarray(full["inputs"][c * BC:(c + 1) * BC])
        in_maps.append(m)

    res = run_bass_kernel_spmd(nc, in_maps, list(range(N_CORES)))
    last_results = res
    probs = np.concatenate([r["probs"] for r in res.results], axis=0)
    abst = np.concatenate([r["abst"] for r in res.results], axis=0)
    hier = np.concatenate([r["hier"] for r in res.results], axis=0)
    return probs, abst, hier


def run_traced(inputs):
    """Profiled run (test-harness helper; requires the axon NTFF hook)."""
    global _cached
    if _cached is None:
        _cached = _build()
    full = {k: np.ascontiguousarray(np.asarray(v, dtype=np.float32))
            for k, v in inputs.items()}
    shared = {k: v for k, v in full.items() if k != "inputs"}
    in_maps = []
    for c in range(N_CORES):
        m = dict(shared)
        m["inputs"] = np.ascontiguousarray(full["inputs"][c * BC:(c + 1) * BC])
        in_maps.append(m)
    return run_bass_kernel_spmd(_cached, in_maps, list(range(N_CORES)), trace=True)
